# revision 1
# baseline (speedup 1.0000x reference)
"""Trainium2 Bass kernel for nn_CrossModalFusion (single-head cross attention).

Per-batch-element cross attention, data-parallel over B=8 across 8 NeuronCores.

Per core (T=2048, D_RGB=400, D_POSE=256, H=512):
    q = rgb @ Wq + bq ; k = pose @ Wk + bk ; v = pose @ Wv
    S = q @ k.T / sqrt(H) ; A = exp(S) (no max-sub needed; scores are O(1))
    y = rgb + bp + bv@Wp + (A @ v) @ Wp / rowsum(A)

Layout strategy (zero on-device transposes):
  - host feeds rgb^T, pose^T (bf16) so projections contract d on partitions
  - qT,kT computed h-major [h,t]; scores computed transposed ST=[tk,tq]
  - exp(ST) on ACT; O^T accumulated via lhsT=v (natural layout), rhs=exp(ST)
  - softmax row-sums via ones-vector matmul; normalization deferred to the
    final projection (row scaling commutes with right-multiplication)
  - all matmul operands bf16 (1 PE cycle/row vs 4 for fp32), fp32 PSUM accum
"""

import sys

if "/opt/trn_rl_repo" not in sys.path:
    sys.path.insert(0, "/opt/trn_rl_repo")

from contextlib import ExitStack

import ml_dtypes
import numpy as np

import concourse.mybir as mybir
import concourse.tile as tile
from concourse import bacc, bass_utils

BF16 = mybir.dt.bfloat16
F32 = mybir.dt.float32
NP_BF16 = ml_dtypes.bfloat16

B, T, DR, DP, H = 8, 2048, 400, 256, 512
PART = 128
TQC = 512                # tq/tk chunk width (max fp32 PSUM free dim)
NCH = T // TQC           # 4 chunks
NTK = T // PART          # 16 key tiles
NHT = H // PART          # 4 h tiles
DR_SPLIT = [128, 128, 128, 16]
DP_SPLIT = [128, 128]
SCALE = float(1.0 / np.sqrt(np.float32(H)))

AT = mybir.ActivationFunctionType
OP = mybir.AluOpType


def build_nc():
    nc = bacc.Bacc(
        "TRN2",
        target_bir_lowering=False,
        debug=False,
        enable_asserts=False,
        num_devices=8,
    )
    xT = nc.dram_tensor("xT", (DR, T), BF16, kind="ExternalInput").ap()
    pT = nc.dram_tensor("pT", (DP, T), BF16, kind="ExternalInput").ap()
    xres = nc.dram_tensor("xres", (T, DR), F32, kind="ExternalInput").ap()
    wq = nc.dram_tensor("wq", (DR, H), BF16, kind="ExternalInput").ap()
    wk = nc.dram_tensor("wk", (DP, H), BF16, kind="ExternalInput").ap()
    wv = nc.dram_tensor("wv", (DP, H), BF16, kind="ExternalInput").ap()
    wp = nc.dram_tensor("wp", (H, DR), BF16, kind="ExternalInput").ap()
    bqc = nc.dram_tensor("bqc", (PART, NHT), F32, kind="ExternalInput").ap()
    bkc = nc.dram_tensor("bkc", (PART, NHT), F32, kind="ExternalInput").ap()
    y = nc.dram_tensor("y", (T, DR), F32, kind="ExternalOutput").ap()

    with tile.TileContext(nc) as tc, ExitStack() as ctx:
        const = ctx.enter_context(tc.tile_pool(name="const", bufs=1))
        mm_ps = ctx.enter_context(tc.tile_pool(name="mm_ps", bufs=2, space="PSUM"))
        ot_ps = ctx.enter_context(tc.tile_pool(name="ot_ps", bufs=4, space="PSUM"))
        sum_ps = ctx.enter_context(tc.tile_pool(name="sum_ps", bufs=1, space="PSUM"))
        y_ps = ctx.enter_context(tc.tile_pool(name="y_ps", bufs=1, space="PSUM"))
        ex_pool = ctx.enter_context(tc.tile_pool(name="ex_pool", bufs=6))
        sums_pool = ctx.enter_context(tc.tile_pool(name="sums_pool", bufs=2))
        recip_pool = ctx.enter_context(tc.tile_pool(name="recip_pool", bufs=2))
        res_pool = ctx.enter_context(tc.tile_pool(name="res_pool", bufs=3))
        ysb_pool = ctx.enter_context(tc.tile_pool(name="ysb_pool", bufs=3))
        scr_pool = ctx.enter_context(tc.tile_pool(name="scr_pool", bufs=2, space="DRAM"))

        # ---- persistent inputs ----
        xt_sb = []
        off = 0
        for di, dsz in enumerate(DR_SPLIT):
            t_ = const.tile([dsz, T], BF16, name=f"xt{di}")
            nc.sync.dma_start(t_[:], xT[off : off + dsz, :])
            xt_sb.append(t_)
            off += dsz
        pt_sb = []
        off = 0
        for di, dsz in enumerate(DP_SPLIT):
            t_ = const.tile([dsz, T], BF16, name=f"pt{di}")
            nc.sync.dma_start(t_[:], pT[off : off + dsz, :])
            pt_sb.append(t_)
            off += dsz
        wq_sb = []
        off = 0
        for di, dsz in enumerate(DR_SPLIT):
            t_ = const.tile([dsz, H], BF16, name=f"wq{di}")
            nc.sync.dma_start(t_[:], wq[off : off + dsz, :])
            wq_sb.append(t_)
            off += dsz
        wk_sb, wv_sb = [], []
        off = 0
        for di, dsz in enumerate(DP_SPLIT):
            t_ = const.tile([dsz, H], BF16, name=f"wk{di}")
            nc.sync.dma_start(t_[:], wk[off : off + dsz, :])
            wk_sb.append(t_)
            t2 = const.tile([dsz, H], BF16, name=f"wv{di}")
            nc.sync.dma_start(t2[:], wv[off : off + dsz, :])
            wv_sb.append(t2)
            off += dsz
        wp_sb = []
        for i in range(NHT):
            t_ = const.tile([PART, DR], BF16, name=f"wp{i}")
            nc.sync.dma_start(t_[:], wp[i * PART : (i + 1) * PART, :])
            wp_sb.append(t_)
        bq_sb = const.tile([PART, NHT], F32, name="bq_sb")
        nc.sync.dma_start(bq_sb[:], bqc[:])
        bk_sb = const.tile([PART, NHT], F32, name="bk_sb")
        nc.sync.dma_start(bk_sb[:], bkc[:])
        ones_sb = const.tile([PART, 1], BF16, name="ones_sb")
        nc.vector.memset(ones_sb[:], 1.0)

        # ---- persistent intermediates ----
        qT_sb = [const.tile([PART, T], BF16, name=f"qT{i}") for i in range(NHT)]
        kT_sb = [const.tile([PART, T], BF16, name=f"kT{i}") for i in range(NHT)]
        v_sb = [const.tile([PART, H], BF16, name=f"v{j}") for j in range(NTK)]
        ot_sb = [const.tile([PART, T], BF16, name=f"ot{i}") for i in range(NHT)]

        # ---- phase B: projections ----
        # qT[h,t] += Wq[d,h].T @ xT[d,t] ; then += bq (per-partition) -> bf16
        for i in range(NHT):
            for c in range(NCH):
                ps = mm_ps.tile([PART, TQC], F32, name=f"qps_{i}_{c}", tag="mmps")
                for di in range(len(DR_SPLIT)):
                    nc.tensor.matmul(
                        ps[:],
                        wq_sb[di][:, i * PART : (i + 1) * PART],
                        xt_sb[di][:, c * TQC : (c + 1) * TQC],
                        start=(di == 0),
                        stop=(di == len(DR_SPLIT) - 1),
                    )
                nc.vector.tensor_scalar_add(
                    qT_sb[i][:, c * TQC : (c + 1) * TQC], ps[:], bq_sb[:, i : i + 1]
                )
        # kT[h,t] = (Wk[d,h].T @ pT[d,t]) * scale + bk*scale -> bf16
        for i in range(NHT):
            for c in range(NCH):
                ps = mm_ps.tile([PART, TQC], F32, name=f"kps_{i}_{c}", tag="mmps")
                for di in range(len(DP_SPLIT)):
                    nc.tensor.matmul(
                        ps[:],
                        wk_sb[di][:, i * PART : (i + 1) * PART],
                        pt_sb[di][:, c * TQC : (c + 1) * TQC],
                        start=(di == 0),
                        stop=(di == len(DP_SPLIT) - 1),
                    )
                nc.vector.tensor_scalar(
                    kT_sb[i][:, c * TQC : (c + 1) * TQC],
                    ps[:],
                    SCALE,
                    bk_sb[:, i : i + 1],
                    op0=OP.mult,
                    op1=OP.add,
                )
        # v[t,h] = pT[d,t].T @ Wv[d,h]  (no bias: bv folded into residual)
        for j in range(NTK):
            ps = mm_ps.tile([PART, H], F32, name=f"vps_{j}", tag="mmps")
            for di in range(len(DP_SPLIT)):
                nc.tensor.matmul(
                    ps[:],
                    pt_sb[di][:, j * PART : (j + 1) * PART],
                    wv_sb[di][:],
                    start=(di == 0),
                    stop=(di == len(DP_SPLIT) - 1),
                )
            nc.vector.tensor_copy(v_sb[j][:], ps[:])

        # ---- phase C: attention, chunked over tq ----
        for c in range(NCH):
            otps = [
                ot_ps.tile([PART, TQC], F32, name=f"otp_{c}_{i}", tag="otp")
                for i in range(NHT)
            ]
            sps = sum_ps.tile([1, TQC], F32, name=f"sump_{c}", tag="sump")
            for j in range(NTK):
                st = mm_ps.tile([PART, TQC], F32, name=f"st_{c}_{j}", tag="mmps")
                for i in range(NHT):
                    nc.tensor.matmul(
                        st[:],
                        kT_sb[i][:, j * PART : (j + 1) * PART],
                        qT_sb[i][:, c * TQC : (c + 1) * TQC],
                        start=(i == 0),
                        stop=(i == NHT - 1),
                    )
                ex = ex_pool.tile([PART, TQC], BF16, name=f"ex_{c}_{j}", tag="ex")
                nc.scalar.activation(ex[:], st[:], AT.Exp)
                for i in range(NHT):
                    nc.tensor.matmul(
                        otps[i][:],
                        v_sb[j][:, i * PART : (i + 1) * PART],
                        ex[:],
                        start=(j == 0),
                        stop=(j == NTK - 1),
                    )
                nc.tensor.matmul(
                    sps[:], ones_sb[:], ex[:], start=(j == 0), stop=(j == NTK - 1)
                )
            for i in range(NHT):
                nc.vector.tensor_copy(ot_sb[i][:, c * TQC : (c + 1) * TQC], otps[i][:])
            sums_sb = sums_pool.tile([1, TQC], F32, name=f"sums_{c}", tag="sums")
            nc.vector.tensor_copy(sums_sb[:], sps[:])
            # bounce [1,512] row through DRAM to get per-partition [128,4] layout
            scr = scr_pool.tile([TQC], F32, name=f"scr_{c}", tag="scr")
            nc.sync.dma_start(scr.rearrange("(a b) -> a b", a=1), sums_sb[:])
            rin = recip_pool.tile([PART, NCH], F32, name=f"rin_{c}", tag="rin")
            nc.sync.dma_start(rin[:], scr.rearrange("(i p) -> p i", p=PART))
            rec = recip_pool.tile([PART, NCH], F32, name=f"rec_{c}", tag="rec")
            nc.vector.reciprocal(rec[:], rin[:])

            # ---- phase D: output projection + normalize + residual ----
            for tl in range(TQC // PART):
                tg = c * (TQC // PART) + tl
                yp = y_ps.tile([PART, DR], F32, name=f"yp_{tg}", tag="yp")
                for i in range(NHT):
                    nc.tensor.matmul(
                        yp[:],
                        ot_sb[i][:, tg * PART : (tg + 1) * PART],
                        wp_sb[i][:],
                        start=(i == 0),
                        stop=(i == NHT - 1),
                    )
                res = res_pool.tile([PART, DR], F32, name=f"res_{tg}", tag="res")
                nc.sync.dma_start(res[:], xres[tg * PART : (tg + 1) * PART, :])
                ysb = ysb_pool.tile([PART, DR], F32, name=f"ysb_{tg}", tag="ysb")
                nc.vector.scalar_tensor_tensor(
                    ysb[:],
                    yp[:],
                    rec[:, tl : tl + 1],
                    res[:],
                    op0=OP.mult,
                    op1=OP.add,
                )
                nc.sync.dma_start(y[tg * PART : (tg + 1) * PART, :], ysb[:])

    nc.compile()
    return nc


_NC_CACHE = None


def get_nc():
    global _NC_CACHE
    if _NC_CACHE is None:
        _NC_CACHE = build_nc()
    return _NC_CACHE


def make_in_maps(rgb, pose, Wq, bq, Wk, bk, Wv, bv, Wp, bp):
    rgb = np.asarray(rgb, np.float32)
    pose = np.asarray(pose, np.float32)
    Wq, bq = np.asarray(Wq, np.float32), np.asarray(bq, np.float32)
    Wk, bk = np.asarray(Wk, np.float32), np.asarray(bk, np.float32)
    Wv, bv = np.asarray(Wv, np.float32), np.asarray(bv, np.float32)
    Wp, bp = np.asarray(Wp, np.float32), np.asarray(bp, np.float32)

    bp_eff = bp + bv @ Wp                       # bv passes through softmax (rows sum to 1)
    xres = rgb + bp_eff                          # residual + all free biases
    xT = np.ascontiguousarray(np.swapaxes(rgb, 1, 2)).astype(NP_BF16)
    pT = np.ascontiguousarray(np.swapaxes(pose, 1, 2)).astype(NP_BF16)
    wq_b = Wq.astype(NP_BF16)
    wk_b = Wk.astype(NP_BF16)
    wv_b = Wv.astype(NP_BF16)
    wp_b = Wp.astype(NP_BF16)
    bqc = np.ascontiguousarray(bq.reshape(NHT, PART).T).astype(np.float32)
    bkc = np.ascontiguousarray((bk * SCALE).reshape(NHT, PART).T).astype(np.float32)
    return [
        dict(
            xT=xT[b], pT=pT[b], xres=xres[b],
            wq=wq_b, wk=wk_b, wv=wv_b, wp=wp_b, bqc=bqc, bkc=bkc,
        )
        for b in range(B)
    ]


def kernel(rgb, pose, Wq, bq, Wk, bk, Wv, bv, Wp, bp):
    in_maps = make_in_maps(rgb, pose, Wq, bq, Wk, bk, Wv, bv, Wp, bp)
    res = bass_utils.run_bass_kernel_spmd(get_nc(), in_maps, core_ids=list(range(B)))
    return np.stack([res.results[b]["y"] for b in range(B)]).astype(np.float32)


# revision 2
# speedup vs baseline: 1.2997x; 1.2997x over previous
"""Trainium2 Bass kernel for nn_CrossModalFusion (single-head cross attention).

Per-batch-element cross attention, data-parallel over B=8 across 8 NeuronCores.

Per core (T=2048, D_RGB=400, D_POSE=256, H=512):
    q = rgb @ Wq + bq ; k = pose @ Wk + bk ; v = pose @ Wv
    S = q @ k.T / sqrt(H) ; A = exp(S) (no max-sub needed; scores are O(1))
    y = rgb + bp + bv@Wp + (A @ v) @ Wp / rowsum(A)

Layout strategy (zero on-device transposes):
  - host feeds rgb^T, pose^T (bf16) so projections contract d on partitions
  - qT,kT computed h-major [h,t]; scores computed transposed ST=[tk,tq]
  - exp(ST) on ACT; O^T accumulated via lhsT=v (natural layout), rhs=exp(ST)
  - softmax row-sums via ones-vector matmul; normalization deferred to the
    final projection (row scaling commutes with right-multiplication)
  - projections use bf16 operands (1 PE cycle/row vs 4 for fp32), the two
    T x T x H attention matmuls use fp8e4m3 + DoubleRow (0.5 cycles/row),
    fp32 PSUM accumulation everywhere
"""

import sys

if "/opt/trn_rl_repo" not in sys.path:
    sys.path.insert(0, "/opt/trn_rl_repo")

from contextlib import ExitStack

import ml_dtypes
import numpy as np

import concourse.mybir as mybir
import concourse.tile as tile
from concourse import bacc, bass_utils

BF16 = mybir.dt.bfloat16
FP8 = mybir.dt.float8e4
F32 = mybir.dt.float32
NP_BF16 = ml_dtypes.bfloat16

B, T, DR, DP, H = 8, 2048, 400, 256, 512
PART = 128
TQC = 512                # tq chunk width (max PSUM free dim)
NCH = T // TQC           # 4 chunks
NTK = T // PART          # 16 key tiles
NKP = NTK // 2           # 8 key tile pairs (DoubleRow)
NHT = H // PART          # 4 h tiles
NHP = NHT // 2           # 2 h tile pairs
DR_SPLIT = [128, 128, 128, 16]
DP_SPLIT = [128, 128]
SCALE = float(1.0 / np.sqrt(np.float32(H)))

AT = mybir.ActivationFunctionType
OP = mybir.AluOpType
DR_MODE = mybir.MatmulPerfMode.DoubleRow


def build_nc():
    nc = bacc.Bacc(
        "TRN2",
        target_bir_lowering=False,
        debug=False,
        enable_asserts=False,
        num_devices=8,
    )
    xT = nc.dram_tensor("xT", (DR, T), BF16, kind="ExternalInput").ap()
    pT = nc.dram_tensor("pT", (DP, T), BF16, kind="ExternalInput").ap()
    xres = nc.dram_tensor("xres", (T, DR), F32, kind="ExternalInput").ap()
    wq = nc.dram_tensor("wq", (DR, H), BF16, kind="ExternalInput").ap()
    wk = nc.dram_tensor("wk", (DP, H), BF16, kind="ExternalInput").ap()
    wv = nc.dram_tensor("wv", (DP, H), BF16, kind="ExternalInput").ap()
    wp = nc.dram_tensor("wp", (H, DR), BF16, kind="ExternalInput").ap()
    bqc = nc.dram_tensor("bqc", (PART, NHT), F32, kind="ExternalInput").ap()
    bkc = nc.dram_tensor("bkc", (PART, NHT), F32, kind="ExternalInput").ap()
    y = nc.dram_tensor("y", (T, DR), F32, kind="ExternalOutput").ap()

    with tile.TileContext(nc) as tc, ExitStack() as ctx:
        const = ctx.enter_context(tc.tile_pool(name="const", bufs=1))
        mm_ps = ctx.enter_context(tc.tile_pool(name="mm_ps", bufs=2, space="PSUM"))
        ot_ps = ctx.enter_context(tc.tile_pool(name="ot_ps", bufs=4, space="PSUM"))
        sum_ps = ctx.enter_context(tc.tile_pool(name="sum_ps", bufs=1, space="PSUM"))
        ex_pool = ctx.enter_context(tc.tile_pool(name="ex_pool", bufs=6))
        sums_pool = ctx.enter_context(tc.tile_pool(name="sums_pool", bufs=2))
        recip_pool = ctx.enter_context(tc.tile_pool(name="recip_pool", bufs=2))
        res_pool = ctx.enter_context(tc.tile_pool(name="res_pool", bufs=3))
        ysb_pool = ctx.enter_context(tc.tile_pool(name="ysb_pool", bufs=3))

        # ---- persistent inputs (DMA order matters: kT path first) ----
        pt_sb = const.tile([PART, len(DP_SPLIT), T], BF16, name="pt_sb")
        nc.sync.dma_start(pt_sb[:], pT.rearrange("(k p) t -> p k t", p=PART))
        wk_sb = const.tile([PART, len(DP_SPLIT), H], BF16, name="wk_sb")
        nc.sync.dma_start(wk_sb[:], wk.rearrange("(k p) h -> p k h", p=PART))
        wq_sb = const.tile([PART, 3, H], BF16, name="wq_sb")
        nc.sync.dma_start(wq_sb[:], wq[: 3 * PART, :].rearrange("(k p) h -> p k h", p=PART))
        wq_tl = const.tile([16, H], BF16, name="wq_tl")
        nc.sync.dma_start(wq_tl[:], wq[3 * PART :, :])
        xt_sb = const.tile([PART, 3, T], BF16, name="xt_sb")
        nc.sync.dma_start(xt_sb[:], xT[: 3 * PART, :].rearrange("(k p) t -> p k t", p=PART))
        xt_tl = const.tile([16, T], BF16, name="xt_tl")
        nc.sync.dma_start(xt_tl[:], xT[3 * PART :, :])
        wv_sb = const.tile([PART, len(DP_SPLIT), H], BF16, name="wv_sb")
        nc.sync.dma_start(wv_sb[:], wv.rearrange("(k p) h -> p k h", p=PART))
        wp_sb = const.tile([PART, NHT, DR], BF16, name="wp_sb")
        nc.sync.dma_start(wp_sb[:], wp.rearrange("(k p) d -> p k d", p=PART))
        bq_sb = const.tile([PART, NHT], F32, name="bq_sb")
        nc.sync.dma_start(bq_sb[:], bqc[:])
        bk_sb = const.tile([PART, NHT], F32, name="bk_sb")
        nc.sync.dma_start(bk_sb[:], bkc[:])
        ones8 = const.tile([PART, 2, 16], FP8, name="ones8")
        nc.vector.memset(ones8[:], 1.0)
        one_f = const.tile([1, 1], F32, name="one_f")
        nc.vector.memset(one_f[:], 1.0)

        def wq_slice(di, i):
            if di < 3:
                return wq_sb[:, di, i * PART : (i + 1) * PART]
            return wq_tl[:, i * PART : (i + 1) * PART]

        def xt_slice(di, c):
            if di < 3:
                return xt_sb[:, di, c * TQC : (c + 1) * TQC]
            return xt_tl[:, c * TQC : (c + 1) * TQC]

        # ---- persistent intermediates (fp8 DoubleRow pair layouts) ----
        # qT8[i2][p, s, t] = q[h = i2*256 + s*128 + p, t]
        qT8 = [const.tile([PART, 2, T], FP8, name=f"qT8_{i}") for i in range(NHP)]
        kT8 = [const.tile([PART, 2, T], FP8, name=f"kT8_{i}") for i in range(NHP)]
        # v8[j2][p, s, h] = v[t = j2*256 + s*128 + p, h]
        v8 = [const.tile([PART, 2, H], FP8, name=f"v8_{j}") for j in range(NKP)]
        ot_sb = [const.tile([PART, T], BF16, name=f"ot{i}") for i in range(NHT)]

        # ---- phase B: projections ----
        # kT[h,t] = (Wk[d,h].T @ pT[d,t]) * scale + bk*scale -> fp8 pair slice
        for i in range(NHT):
            for c in range(NCH):
                ps = mm_ps.tile([PART, TQC], F32, name=f"kps_{i}_{c}", tag="mmps")
                for di in range(len(DP_SPLIT)):
                    nc.tensor.matmul(
                        ps[:],
                        wk_sb[:, di, i * PART : (i + 1) * PART],
                        pt_sb[:, di, c * TQC : (c + 1) * TQC],
                        start=(di == 0),
                        stop=(di == len(DP_SPLIT) - 1),
                    )
                nc.vector.tensor_scalar(
                    kT8[i // 2][:, i % 2, c * TQC : (c + 1) * TQC],
                    ps[:],
                    SCALE,
                    bk_sb[:, i : i + 1],
                    op0=OP.mult,
                    op1=OP.add,
                )
        # qT[h,t] = Wq[d,h].T @ xT[d,t] + bq -> fp8 pair slice
        for i in range(NHT):
            for c in range(NCH):
                ps = mm_ps.tile([PART, TQC], F32, name=f"qps_{i}_{c}", tag="mmps")
                for di in range(len(DR_SPLIT)):
                    nc.tensor.matmul(
                        ps[:],
                        wq_slice(di, i),
                        xt_slice(di, c),
                        start=(di == 0),
                        stop=(di == len(DR_SPLIT) - 1),
                    )
                nc.vector.tensor_scalar_add(
                    qT8[i // 2][:, i % 2, c * TQC : (c + 1) * TQC],
                    ps[:],
                    bq_sb[:, i : i + 1],
                )
        # v[t,h] = pT[d,t].T @ Wv[d,h] -> fp8 pair slice (bv folded into residual)
        for j in range(NTK):
            ps = mm_ps.tile([PART, H], F32, name=f"vps_{j}", tag="mmps")
            for di in range(len(DP_SPLIT)):
                nc.tensor.matmul(
                    ps[:],
                    pt_sb[:, di, j * PART : (j + 1) * PART],
                    wv_sb[:, di, :],
                    start=(di == 0),
                    stop=(di == len(DP_SPLIT) - 1),
                )
            nc.scalar.copy(v8[j // 2][:, j % 2, :], ps[:])

        # ---- phase C: attention, chunked over tq ----
        for c in range(NCH):
            otps = [
                ot_ps.tile([PART, TQC], F32, name=f"otp_{c}_{i}", tag="otp")
                for i in range(NHT)
            ]
            sps = sum_ps.tile([1, TQC], F32, name=f"sump_{c}", tag="sump")
            exs = []
            for j in range(NTK):
                st = mm_ps.tile([PART, TQC], F32, name=f"st_{c}_{j}", tag="mmps")
                for i2 in range(NHP):
                    nc.tensor.matmul(
                        st[:],
                        kT8[i2][:, :, j * PART : (j + 1) * PART],
                        qT8[i2][:, :, c * TQC : (c + 1) * TQC],
                        start=(i2 == 0),
                        stop=(i2 == NHP - 1),
                        perf_mode=DR_MODE,
                    )
                if j % 2 == 0:
                    ex = ex_pool.tile([PART, 2, TQC], FP8, name=f"ex_{c}_{j}", tag="ex")
                    exs.append(ex)
                nc.scalar.activation(exs[-1][:, j % 2, :], st[:], AT.Exp)
                if j % 2 == 1:
                    j2 = j // 2
                    ex = exs[-1]
                    for i in range(NHT):
                        nc.tensor.matmul(
                            otps[i][:],
                            v8[j2][:, :, i * PART : (i + 1) * PART],
                            ex[:],
                            start=(j2 == 0),
                            stop=(j2 == NKP - 1),
                            perf_mode=DR_MODE,
                        )
                    nc.tensor.matmul(
                        sps[:],
                        ones8[:, :, 0:1],
                        ex[:],
                        start=(j2 == 0),
                        stop=(j2 == NKP - 1),
                        perf_mode=DR_MODE,
                    )
            for i in range(NHT):
                nc.vector.tensor_copy(ot_sb[i][:, c * TQC : (c + 1) * TQC], otps[i][:])
            sums_sb = sums_pool.tile([1, TQC], F32, name=f"sums_{c}", tag="sums")
            nc.vector.tensor_copy(sums_sb[:], sps[:])
            # transpose the [1,512] row of sums into [128,4] via 4 tiny K=1 matmuls
            scol = sum_ps.tile([PART, NCH], F32, name=f"scol_{c}", tag="scol")
            for tl in range(TQC // PART):
                nc.tensor.matmul(
                    scol[:, tl : tl + 1],
                    sums_sb[0:1, tl * PART : (tl + 1) * PART],
                    one_f[:],
                    start=True,
                    stop=True,
                )
            rec = recip_pool.tile([PART, NCH], F32, name=f"rec_{c}", tag="rec")
            nc.vector.reciprocal(rec[:], scol[:])

            # ---- phase D: output projection + normalize + residual ----
            for tl in range(TQC // PART):
                tg = c * (TQC // PART) + tl
                yp = mm_ps.tile([PART, DR], F32, name=f"yp_{tg}", tag="mmps")
                for i in range(NHT):
                    nc.tensor.matmul(
                        yp[:],
                        ot_sb[i][:, tg * PART : (tg + 1) * PART],
                        wp_sb[:, i, :],
                        start=(i == 0),
                        stop=(i == NHT - 1),
                    )
                res = res_pool.tile([PART, DR], F32, name=f"res_{tg}", tag="res")
                nc.gpsimd.dma_start(res[:], xres[tg * PART : (tg + 1) * PART, :])
                ysb = ysb_pool.tile([PART, DR], F32, name=f"ysb_{tg}", tag="ysb")
                nc.vector.scalar_tensor_tensor(
                    ysb[:],
                    yp[:],
                    rec[:, tl : tl + 1],
                    res[:],
                    op0=OP.mult,
                    op1=OP.add,
                )
                nc.gpsimd.dma_start(y[tg * PART : (tg + 1) * PART, :], ysb[:])

    nc.compile()
    return nc


_NC_CACHE = None


def get_nc():
    global _NC_CACHE
    if _NC_CACHE is None:
        _NC_CACHE = build_nc()
    return _NC_CACHE


def make_in_maps(rgb, pose, Wq, bq, Wk, bk, Wv, bv, Wp, bp):
    rgb = np.asarray(rgb, np.float32)
    pose = np.asarray(pose, np.float32)
    Wq, bq = np.asarray(Wq, np.float32), np.asarray(bq, np.float32)
    Wk, bk = np.asarray(Wk, np.float32), np.asarray(bk, np.float32)
    Wv, bv = np.asarray(Wv, np.float32), np.asarray(bv, np.float32)
    Wp, bp = np.asarray(Wp, np.float32), np.asarray(bp, np.float32)

    bp_eff = bp + bv @ Wp                       # bv passes through softmax (rows sum to 1)
    xres = rgb + bp_eff                          # residual + all free biases
    xT = np.ascontiguousarray(np.swapaxes(rgb, 1, 2)).astype(NP_BF16)
    pT = np.ascontiguousarray(np.swapaxes(pose, 1, 2)).astype(NP_BF16)
    wq_b = Wq.astype(NP_BF16)
    wk_b = Wk.astype(NP_BF16)
    wv_b = Wv.astype(NP_BF16)
    wp_b = Wp.astype(NP_BF16)
    bqc = np.ascontiguousarray(bq.reshape(NHT, PART).T).astype(np.float32)
    bkc = np.ascontiguousarray((bk * SCALE).reshape(NHT, PART).T).astype(np.float32)
    return [
        dict(
            xT=xT[b], pT=pT[b], xres=xres[b],
            wq=wq_b, wk=wk_b, wv=wv_b, wp=wp_b, bqc=bqc, bkc=bkc,
        )
        for b in range(B)
    ]


def kernel(rgb, pose, Wq, bq, Wk, bk, Wv, bv, Wp, bp):
    in_maps = make_in_maps(rgb, pose, Wq, bq, Wk, bk, Wv, bv, Wp, bp)
    res = bass_utils.run_bass_kernel_spmd(get_nc(), in_maps, core_ids=list(range(B)))
    return np.stack([res.results[b]["y"] for b in range(B)]).astype(np.float32)
